# revision 19
# baseline (speedup 1.0000x reference)
"""Trainium2 Bass kernel for nn_FFDNN (4-layer functional FFN).

Math: 4 layers of H[i,k,q] = B[k,q] + sum_{j,p} (h[i,j,p] * tw[p]) * W[j,k,p,q]
(ReLU between layers) == plain GEMMs with contraction dim (j,p) and output
dim (k,q) once the trapezoid weights tw are folded into W (host-side).

GEMM shapes: L0 [256,256]@[256,8192], L1/L2 [256,8192]@[8192,8192],
L3 [256,8192]@[8192,256].  Weights dominate (528 MiB) -> shard L1/L2/L3
tensor-parallel over the node dim k across 8 cores (66 MiB/core, the
minimum possible weight traffic; the kernel is DMA-bound).  L0 is tiny and
fully replicated on every core, which removes one all-gather entirely.
One all-gather boundary remains (H1): L1 computes its local output shard
in two column halves so the first AG launches while the second half is
still computing; L2 consumes the two gathered halves in order.  L3 is
sharded over its contraction dim j (= k of L2), each core emits a partial
[256,256] output and the host sums the 8 partials.

Activations stay feature-major [feature, batch] end to end -> no
transposes anywhere.  Bias+ReLU ride along the mandatory PSUM-drain op
(alternating ScalarE/VectorE so the drain matches the matmul rate).
Weights are pre-arranged on the host so every weight DMA is a single
1-2 MiB transfer with >=8 KiB contiguous runs per partition.

Matmul dtype knob (KERNEL_MM_DT): float32r (TF32, default: 1 cyc/row at
fp32 storage, measured rel err ~3e-4), float32 (exact, 4 cyc/row),
bfloat16 (half traffic, rel err ~5e-3).
"""

import contextlib
import ctypes
import os
import sys
import types

sys.path.insert(0, "/opt/trn_rl_repo")

import numpy as np

# ---------------------------------------------------------------- constants
R = 8  # cores / ranks
B = 256  # batch
C, QI, QH, K = 4, 64, 128, 64
F_IN = C * QI  # 256   L0 contraction
F_HID = K * QH  # 8192  hidden width
F_OUT = C * QI  # 256   final output features
NCH = F_HID // 128  # 64 contraction chunks in the big layers
SHARD = F_HID // R  # 1024 features per core for L1/L2 outputs
M_LOC = SHARD // 128  # 8 local 128-row feature chunks
N_AG = 4  # all-gather split for H1 (one per L1 column quarter)
M_GRP = M_LOC // N_AG  # 2
DX_IN = 1.0 / (QI - 1)
DX_HID = 1.0 / (QH - 1)

_CACHE = {}
LAST_EXEC_NS = None
LAST_RESULT = None


def _install_ntff_hook():
    """Provide antenv.axon_hooks (missing in this image) so trace=True works."""
    if "antenv.axon_hooks" in sys.modules:
        return
    try:
        lib = ctypes.CDLL("/opt/axon/libaxon_pjrt.so")
        if not hasattr(lib, "axon_start_nrt_profile"):
            return
        lib.axon_start_nrt_profile.argtypes = [
            ctypes.POINTER(ctypes.c_int64),
            ctypes.c_size_t,
        ]
        lib.axon_start_nrt_profile.restype = ctypes.c_int64
        lib.axon_stop_nrt_profile.argtypes = [ctypes.c_char_p]
        lib.axon_stop_nrt_profile.restype = ctypes.c_int64

        @contextlib.contextmanager
        def _hook(output_dir, device_ids):
            import jax

            jax.devices()
            if device_ids:
                ids = (ctypes.c_int64 * len(device_ids))(*device_ids)
                rc = lib.axon_start_nrt_profile(ids, len(device_ids))
            else:
                rc = lib.axon_start_nrt_profile(None, 0)
            if rc != 0:
                raise RuntimeError(f"axon_start_nrt_profile rc={rc}")
            try:
                yield
            finally:
                n = lib.axon_stop_nrt_profile(str(output_dir).encode())
                print(f"profile: {n} ntff file(s) -> {output_dir}", file=sys.stderr)

        mod = types.ModuleType("antenv.axon_hooks")
        mod.get_axon_ntff_profile_hook = lambda: _hook
        sys.modules["antenv.axon_hooks"] = mod
    except Exception as e:  # tracing is optional
        print(f"ntff hook unavailable: {e}", file=sys.stderr)


def _gh1_order():
    """Row order of the gathered H1 halves: [h, r, mi, q] -> k*QH+q."""
    h = np.arange(N_AG)[:, None, None]
    r = np.arange(R)[None, :, None]
    mi = np.arange(M_GRP)[None, None, :]
    ks = (r * M_LOC + h * M_GRP + mi).reshape(-1)  # k per 128-row chunk
    return (ks[:, None] * QH + np.arange(QH)[None, :]).reshape(-1)


def _build_nc(mm_name):
    import concourse.mybir as mybir
    import concourse.tile as tile
    from concourse import bacc

    dt = mybir.dt
    mdt = getattr(dt, mm_name)
    f32 = dt.float32
    Relu = mybir.ActivationFunctionType.Relu
    RG = [list(range(R))]

    nc = bacc.Bacc(None, target_bir_lowering=False)

    # host-pre-arranged inputs; every weight DMA lands [128, F] with
    # per-partition-contiguous runs
    xT = nc.dram_tensor("xT", [2, 128, B], mdt, kind="ExternalInput")
    w0 = nc.dram_tensor("w0", [8, 128, 2048], mdt, kind="ExternalInput")
    w1 = nc.dram_tensor("w1", [N_AG, 4, 128, 4096], mdt, kind="ExternalInput")
    w2 = nc.dram_tensor("w2", [16, 128, 4096], mdt, kind="ExternalInput")
    w3 = nc.dram_tensor("w3", [128, M_LOC * F_OUT], mdt, kind="ExternalInput")
    b0 = nc.dram_tensor("b0", [128, K], f32, kind="ExternalInput")
    b1 = nc.dram_tensor("b1", [128, M_LOC], f32, kind="ExternalInput")
    b2 = nc.dram_tensor("b2", [128, M_LOC], f32, kind="ExternalInput")
    b3 = nc.dram_tensor("b3", [128, 2], f32, kind="ExternalInput")
    outp = nc.dram_tensor("out", [2, 128, B], f32, kind="ExternalOutput")

    def drain(out_ap, ps_ap, bias_ap, idx):
        """PSUM drain with fused bias+ReLU, alternating ScalarE/VectorE."""
        if idx % 2 == 0:
            nc.scalar.activation(out_ap, ps_ap, Relu, bias=bias_ap)
        else:
            nc.vector.tensor_scalar(
                out=out_ap,
                in0=ps_ap,
                scalar1=bias_ap,
                scalar2=0.0,
                op0=mybir.AluOpType.add,
                op1=mybir.AluOpType.max,
            )

    with tile.TileContext(nc) as tc:
        with (
            tc.tile_pool(name="wt", bufs=4) as wpool,
            tc.tile_pool(name="ht", bufs=6) as hpool,
            tc.tile_pool(name="h0", bufs=NCH + 2) as h0pool,
            tc.tile_pool(name="hsm", bufs=6) as spool,
            tc.tile_pool(name="h2", bufs=M_LOC) as h2pool,
            tc.tile_pool(name="misc", bufs=1) as mpool,
            tc.tile_pool(name="ps", bufs=8, space="PSUM") as pspool,
            tc.tile_pool(name="dram", bufs=1, space="DRAM") as dpool,
        ):
            # biases + x
            b0t = mpool.tile([128, K], f32, tag="b0t")
            b1t = mpool.tile([128, M_LOC], f32, tag="b1t")
            b2t = mpool.tile([128, M_LOC], f32, tag="b2t")
            b3t = mpool.tile([128, 2], f32, tag="b3t")
            nc.sync.dma_start(out=b0t[:], in_=b0[:])
            nc.sync.dma_start(out=b1t[:], in_=b1[:])
            nc.sync.dma_start(out=b2t[:], in_=b2[:])
            nc.sync.dma_start(out=b3t[:], in_=b3[:])
            x_t = mpool.tile([128, 2, B], mdt, tag="x")
            for cc in range(2):
                nc.sync.dma_start(out=x_t[:, cc, :], in_=xT[cc])
            w3t = mpool.tile([128, M_LOC * F_OUT], mdt, tag="w3t")
            nc.sync.dma_start(out=w3t[:], in_=w3[:])

            # DRAM bounce buffers for the H1 all-gather halves; AG outputs
            # must be addr_space="Shared" (remote SDMA writes land there)
            hb1 = [
                dpool.tile([M_GRP * 128, B], mdt, tag=f"hb1_{h}", name=f"hb1_{h}")
                for h in range(N_AG)
            ]
            gh1 = [
                dpool.tile(
                    [R * M_GRP * 128, B],
                    mdt,
                    tag=f"gh1_{h}",
                    name=f"gh1_{h}",
                    addr_space="Shared",
                )
                for h in range(N_AG)
            ]

            h0_loc = []  # 64 chunks [128, B]: full H0 (replicated compute)
            h2_loc = []  # 8 chunks: local H2 shard

            # ---- L0: replicated; 8 waves x (8 out-chunks x 2-step contraction)
            with nc.named_scope("L0"):
                for wave in range(8):
                    wt = wpool.tile([128, 2048], mdt, tag="wt", name="wt0")
                    nc.sync.dma_start(out=wt[:], in_=w0[wave])
                    ps = [
                        pspool.tile([128, B], f32, tag="ps", name=f"ps0_{wave}_{i}")
                        for i in range(8)
                    ]
                    for m8 in range(8):
                        for cc in range(2):
                            nc.tensor.matmul(
                                ps[m8][:],
                                wt[:, cc * 1024 + m8 * 128 : cc * 1024 + (m8 + 1) * 128],
                                x_t[:, cc, :],
                                start=(cc == 0),
                                stop=(cc == 1),
                            )
                    for m8 in range(8):
                        k = wave * 8 + m8
                        h = h0pool.tile([128, B], mdt, tag="h0", name="h0c")
                        drain(h[:], ps[m8][:], b0t[:, k : k + 1], m8)
                        h0_loc.append(h)

            # ---- L1: contract 64 chunks (SBUF h0) in four column quarters
            # so the all-gathers fire at 25/50/75/100% of L1 and overlap the
            # remaining quarters' compute
            with nc.named_scope("L1"):
                for hh in range(N_AG):
                    ps1 = [
                        pspool.tile([128, B], f32, tag="ps", name=f"ps1_{hh}_{i}")
                        for i in range(M_GRP)
                    ]
                    for sc in range(4):
                        wt = wpool.tile([128, 4096], mdt, tag="wt", name="wt1")
                        nc.sync.dma_start(out=wt[:], in_=w1[hh, sc])
                        for c16 in range(16):
                            c = sc * 16 + c16
                            for m in range(M_GRP):
                                nc.tensor.matmul(
                                    ps1[m][:],
                                    wt[:, c16 * 256 + m * 128 : c16 * 256 + (m + 1) * 128],
                                    h0_loc[c][:],
                                    start=(c == 0),
                                    stop=(c == NCH - 1),
                                )
                    for mi in range(M_GRP):
                        m = hh * M_GRP + mi
                        h = spool.tile([128, B], mdt, tag="h1", name=f"h1_{m}")
                        drain(h[:], ps1[mi][:], b1t[:, m : m + 1], mi)
                        nc.gpsimd.dma_start(
                            out=hb1[hh][mi * 128 : (mi + 1) * 128, :], in_=h[:]
                        )
                    nc.gpsimd.collective_compute(
                        "AllGather",
                        mybir.AluOpType.bypass,
                        ins=[hb1[hh][:]],
                        outs=[gh1[hh][:]],
                        replica_groups=RG,
                    )

            # ---- L2: contract 64 chunks (gathered H1), output local shard ----
            with nc.named_scope("L2"):
                ps2 = [
                    pspool.tile([128, B], f32, tag="ps", name=f"ps2_{i}")
                    for i in range(M_LOC)
                ]
                for sc in range(16):
                    wt2 = wpool.tile([128, 4096], mdt, tag="wt", name="wt2")
                    nc.sync.dma_start(out=wt2[:], in_=w2[sc])
                    hh, s_in = divmod(sc, 16 // N_AG)
                    ht = hpool.tile([128, 4, B], mdt, tag="ht", name="ht")
                    nc.gpsimd.dma_start(
                        out=ht[:],
                        in_=gh1[hh][s_in * 512 : (s_in + 1) * 512, :].rearrange(
                            "(c p) n -> p c n", p=128
                        ),
                    )
                    for c4 in range(4):
                        c = sc * 4 + c4
                        for m in range(M_LOC):
                            nc.tensor.matmul(
                                ps2[m][:],
                                wt2[:, c4 * 1024 + m * 128 : c4 * 1024 + (m + 1) * 128],
                                ht[:, c4, :],
                                start=(c == 0),
                                stop=(c == NCH - 1),
                            )
                for m in range(M_LOC):
                    h = h2pool.tile([128, B], mdt, tag="h2", name=f"h2_{m}")
                    drain(h[:], ps2[m][:], b2t[:, m : m + 1], m)
                    h2_loc.append(h)

            # ---- L3: contract local 8 chunks, output partial [256, B] ----
            with nc.named_scope("L3"):
                ps3 = [
                    pspool.tile([128, B], f32, tag="ps", name=f"ps3_{i}")
                    for i in range(2)
                ]
                for cx in range(M_LOC):
                    for mo in range(2):
                        nc.tensor.matmul(
                            ps3[mo][:],
                            w3t[:, cx * F_OUT + mo * 128 : cx * F_OUT + (mo + 1) * 128],
                            h2_loc[cx][:],
                            start=(cx == 0),
                            stop=(cx == M_LOC - 1),
                        )
                for mo in range(2):
                    ot = spool.tile([128, B], f32, tag="ot", name=f"ot_{mo}")
                    nc.vector.tensor_scalar(
                        out=ot[:],
                        in0=ps3[mo][:],
                        scalar1=b3t[:, mo : mo + 1],
                        scalar2=None,
                        op0=mybir.AluOpType.add,
                    )
                    nc.sync.dma_start(out=outp[mo], in_=ot[:])

    nc.compile()
    return nc


def _trap(n, dx):
    w = np.ones(n, np.float64)
    w[0] = 0.5
    w[-1] = 0.5
    return (w * dx).astype(np.float32)


def _prep_inputs(inputs, mm_name):
    if mm_name == "bfloat16":
        import ml_dtypes

        mm_np = ml_dtypes.bfloat16
    else:
        mm_np = np.float32

    x = np.asarray(inputs["x"], np.float32)
    W0 = np.asarray(inputs["W0"], np.float32)
    W1 = np.asarray(inputs["W1"], np.float32)
    W2 = np.asarray(inputs["W2"], np.float32)
    W3 = np.asarray(inputs["W3"], np.float32)
    B0 = np.asarray(inputs["B0"], np.float32)
    B1 = np.asarray(inputs["B1"], np.float32)
    B2 = np.asarray(inputs["B2"], np.float32)
    B3 = np.asarray(inputs["B3"], np.float32)
    tw_in = _trap(QI, DX_IN)
    tw_h = _trap(QH, DX_HID)

    # x -> feature-major [(j p), i], chunked [2, 128, B]
    xT = np.ascontiguousarray(x.reshape(B, F_IN).T).reshape(2, 128, B).astype(mm_np)

    # L0 (replicated): [wave, p, (c f1024)] -> per-partition 8 KiB runs
    Wn0 = (W0 * tw_in[None, None, :, None]).transpose(0, 2, 1, 3).reshape(F_IN, F_HID)
    w0_dev = np.ascontiguousarray(
        Wn0.reshape(2, 128, 8, 1024).transpose(2, 1, 0, 3).reshape(8, 128, 2048)
    ).astype(mm_np)

    order1 = _gh1_order()

    def hidden_mat(W):  # [j,k,p,q] -> [(j p), (k q)] with tw_h folded into rows
        return (W * tw_h[None, None, :, None]).transpose(0, 2, 1, 3).reshape(F_HID, -1)

    Wn1 = hidden_mat(W1)
    Wn2 = hidden_mat(W2)[order1]  # rows follow the gathered-H1 ordering
    Wn3 = hidden_mat(W3)

    per_rank = []
    for r in range(R):
        cols = slice(r * SHARD, (r + 1) * SHARD)  # k in [r*8, r*8+8) natural
        # L1: [quarter, sc4, p, (c16 f256)]; rows (c p), cols (quarter f256)
        w1r = Wn1[:, cols].reshape(4, 16, 128, N_AG, 256)  # [sc, c16, p, q, f]
        w1_dev = np.ascontiguousarray(
            w1r.transpose(3, 0, 2, 1, 4).reshape(N_AG, 4, 128, 4096)
        ).astype(mm_np)
        # L2: [sc16, p, (c4 f1024)]
        w2r = Wn2[:, cols].reshape(16, 4, 128, 1024)  # [sc, c4, p, f]
        w2_dev = np.ascontiguousarray(
            w2r.transpose(0, 2, 1, 3).reshape(16, 128, 4096)
        ).astype(mm_np)
        # L3: rows j in own shard -> [p, (c F_OUT)]
        w3r = Wn3[r * SHARD : (r + 1) * SHARD].reshape(M_LOC, 128, F_OUT)
        w3_dev = np.ascontiguousarray(
            w3r.transpose(1, 0, 2).reshape(128, M_LOC * F_OUT)
        ).astype(mm_np)
        per_rank.append(
            {
                "xT": xT,
                "w0": w0_dev,
                "w1": w1_dev,
                "w2": w2_dev,
                "w3": w3_dev,
                "b0": np.ascontiguousarray(B0.T),
                "b1": np.ascontiguousarray(B1[r * M_LOC : (r + 1) * M_LOC].T),
                "b2": np.ascontiguousarray(B2[r * M_LOC : (r + 1) * M_LOC].T),
                "b3": np.ascontiguousarray((B3.reshape(F_OUT) / R).reshape(2, 128).T),
            }
        )
    return per_rank


def kernel(**inputs):
    global LAST_EXEC_NS, LAST_RESULT
    mm_name = os.environ.get("KERNEL_MM_DT", "float32r")
    trace = os.environ.get("KERNEL_TRACE", "0") == "1"
    if trace:
        _install_ntff_hook()

    from concourse.bass_utils import run_bass_kernel_spmd

    nc = _CACHE.get(mm_name)
    if nc is None:
        nc = _CACHE[mm_name] = _build_nc(mm_name)

    in_maps = _prep_inputs(inputs, mm_name)
    res = run_bass_kernel_spmd(nc, in_maps, core_ids=list(range(R)), trace=trace)
    LAST_EXEC_NS = res.exec_time_ns
    LAST_RESULT = res

    total = np.zeros((2, 128, B), np.float32)
    for r in range(R):
        total += np.asarray(res.results[r]["out"], np.float32)
    # rows are output features (c*QI + q); columns are batch
    return np.ascontiguousarray(
        total.reshape(F_OUT, B).reshape(C, QI, B).transpose(2, 0, 1)
    )


# revision 20
# speedup vs baseline: 1.0092x; 1.0092x over previous
"""Trainium2 Bass kernel for nn_FFDNN (4-layer functional FFN).

Math: 4 layers of H[i,k,q] = B[k,q] + sum_{j,p} (h[i,j,p] * tw[p]) * W[j,k,p,q]
(ReLU between layers) == plain GEMMs with contraction dim (j,p) and output
dim (k,q) once the trapezoid weights tw are folded into W (host-side).

GEMM shapes: L0 [256,256]@[256,8192], L1/L2 [256,8192]@[8192,8192],
L3 [256,8192]@[8192,256].  Weights dominate (528 MiB) -> shard L1/L2/L3
tensor-parallel over the node dim k across 8 cores (66 MiB/core, the
minimum possible weight traffic; the kernel is DMA-bound).  L0 is tiny and
fully replicated on every core, which removes one all-gather entirely.
One all-gather boundary remains (H1): L1 computes its local output shard
in two column halves so the first AG launches while the second half is
still computing; L2 consumes the two gathered halves in order.  L3 is
sharded over its contraction dim j (= k of L2), each core emits a partial
[256,256] output and the host sums the 8 partials.

Activations stay feature-major [feature, batch] end to end -> no
transposes anywhere.  Bias+ReLU ride along the mandatory PSUM-drain op
(alternating ScalarE/VectorE so the drain matches the matmul rate).
Weights are pre-arranged on the host so every weight DMA is a single
1-2 MiB transfer with >=8 KiB contiguous runs per partition.

Matmul dtype knob (KERNEL_MM_DT): float32r (TF32, default: 1 cyc/row at
fp32 storage, measured rel err ~3e-4), float32 (exact, 4 cyc/row),
bfloat16 (half traffic, rel err ~5e-3).
"""

import contextlib
import ctypes
import os
import sys
import types

sys.path.insert(0, "/opt/trn_rl_repo")

import numpy as np

# ---------------------------------------------------------------- constants
R = 8  # cores / ranks
B = 256  # batch
C, QI, QH, K = 4, 64, 128, 64
F_IN = C * QI  # 256   L0 contraction
F_HID = K * QH  # 8192  hidden width
F_OUT = C * QI  # 256   final output features
NCH = F_HID // 128  # 64 contraction chunks in the big layers
SHARD = F_HID // R  # 1024 features per core for L1/L2 outputs
M_LOC = SHARD // 128  # 8 local 128-row feature chunks
N_AG = 4  # all-gather split for H1 (one per L1 column quarter)
M_GRP = M_LOC // N_AG  # 2
DX_IN = 1.0 / (QI - 1)
DX_HID = 1.0 / (QH - 1)

_CACHE = {}
LAST_EXEC_NS = None
LAST_RESULT = None


def _install_ntff_hook():
    """Provide antenv.axon_hooks (missing in this image) so trace=True works."""
    if "antenv.axon_hooks" in sys.modules:
        return
    try:
        lib = ctypes.CDLL("/opt/axon/libaxon_pjrt.so")
        if not hasattr(lib, "axon_start_nrt_profile"):
            return
        lib.axon_start_nrt_profile.argtypes = [
            ctypes.POINTER(ctypes.c_int64),
            ctypes.c_size_t,
        ]
        lib.axon_start_nrt_profile.restype = ctypes.c_int64
        lib.axon_stop_nrt_profile.argtypes = [ctypes.c_char_p]
        lib.axon_stop_nrt_profile.restype = ctypes.c_int64

        @contextlib.contextmanager
        def _hook(output_dir, device_ids):
            import jax

            jax.devices()
            if device_ids:
                ids = (ctypes.c_int64 * len(device_ids))(*device_ids)
                rc = lib.axon_start_nrt_profile(ids, len(device_ids))
            else:
                rc = lib.axon_start_nrt_profile(None, 0)
            if rc != 0:
                raise RuntimeError(f"axon_start_nrt_profile rc={rc}")
            try:
                yield
            finally:
                n = lib.axon_stop_nrt_profile(str(output_dir).encode())
                print(f"profile: {n} ntff file(s) -> {output_dir}", file=sys.stderr)

        mod = types.ModuleType("antenv.axon_hooks")
        mod.get_axon_ntff_profile_hook = lambda: _hook
        sys.modules["antenv.axon_hooks"] = mod
    except Exception as e:  # tracing is optional
        print(f"ntff hook unavailable: {e}", file=sys.stderr)


def _gh1_order():
    """Row order of the gathered H1 halves: [h, r, mi, q] -> k*QH+q."""
    h = np.arange(N_AG)[:, None, None]
    r = np.arange(R)[None, :, None]
    mi = np.arange(M_GRP)[None, None, :]
    ks = (r * M_LOC + h * M_GRP + mi).reshape(-1)  # k per 128-row chunk
    return (ks[:, None] * QH + np.arange(QH)[None, :]).reshape(-1)


def _build_nc(mm_name):
    import concourse.mybir as mybir
    import concourse.tile as tile
    from concourse import bacc

    dt = mybir.dt
    mdt = getattr(dt, mm_name)
    f32 = dt.float32
    Relu = mybir.ActivationFunctionType.Relu
    RG = [list(range(R))]

    nc = bacc.Bacc(None, target_bir_lowering=False)

    # host-pre-arranged inputs; every weight DMA lands [128, F] with
    # per-partition-contiguous runs
    xT = nc.dram_tensor("xT", [2, 128, B], mdt, kind="ExternalInput")
    w0 = nc.dram_tensor("w0", [8, 128, 2048], mdt, kind="ExternalInput")
    w1 = nc.dram_tensor("w1", [N_AG, 4, 128, 4096], mdt, kind="ExternalInput")
    w2 = nc.dram_tensor("w2", [16, 128, 4096], mdt, kind="ExternalInput")
    w3 = nc.dram_tensor("w3", [128, M_LOC * F_OUT], mdt, kind="ExternalInput")
    b0 = nc.dram_tensor("b0", [128, K], f32, kind="ExternalInput")
    b1 = nc.dram_tensor("b1", [128, M_LOC], f32, kind="ExternalInput")
    b2 = nc.dram_tensor("b2", [128, M_LOC], f32, kind="ExternalInput")
    b3 = nc.dram_tensor("b3", [128, 2], f32, kind="ExternalInput")
    outp = nc.dram_tensor("out", [2, 128, B], f32, kind="ExternalOutput")

    def drain(out_ap, ps_ap, bias_ap, idx):
        """PSUM drain with fused bias+ReLU, alternating ScalarE/VectorE."""
        if idx % 2 == 0:
            nc.scalar.activation(out_ap, ps_ap, Relu, bias=bias_ap)
        else:
            nc.vector.tensor_scalar(
                out=out_ap,
                in0=ps_ap,
                scalar1=bias_ap,
                scalar2=0.0,
                op0=mybir.AluOpType.add,
                op1=mybir.AluOpType.max,
            )

    with tile.TileContext(nc) as tc:
        with (
            tc.tile_pool(name="wt", bufs=5) as wpool,
            tc.tile_pool(name="ht", bufs=4) as hpool,
            tc.tile_pool(name="h0", bufs=NCH + 2) as h0pool,
            tc.tile_pool(name="hsm", bufs=6) as spool,
            tc.tile_pool(name="h2", bufs=M_LOC) as h2pool,
            tc.tile_pool(name="misc", bufs=1) as mpool,
            tc.tile_pool(name="ps", bufs=8, space="PSUM") as pspool,
            tc.tile_pool(name="dram", bufs=1, space="DRAM") as dpool,
        ):
            # biases + x
            b0t = mpool.tile([128, K], f32, tag="b0t")
            b1t = mpool.tile([128, M_LOC], f32, tag="b1t")
            b2t = mpool.tile([128, M_LOC], f32, tag="b2t")
            b3t = mpool.tile([128, 2], f32, tag="b3t")
            nc.sync.dma_start(out=b0t[:], in_=b0[:])
            nc.sync.dma_start(out=b1t[:], in_=b1[:])
            nc.sync.dma_start(out=b2t[:], in_=b2[:])
            nc.sync.dma_start(out=b3t[:], in_=b3[:])
            x_t = mpool.tile([128, 2, B], mdt, tag="x")
            for cc in range(2):
                nc.sync.dma_start(out=x_t[:, cc, :], in_=xT[cc])
            w3t = mpool.tile([128, M_LOC * F_OUT], mdt, tag="w3t")
            nc.sync.dma_start(out=w3t[:], in_=w3[:])

            # DRAM bounce buffers for the H1 all-gather halves; AG outputs
            # must be addr_space="Shared" (remote SDMA writes land there)
            hb1 = [
                dpool.tile([M_GRP * 128, B], mdt, tag=f"hb1_{h}", name=f"hb1_{h}")
                for h in range(N_AG)
            ]
            gh1 = [
                dpool.tile(
                    [R * M_GRP * 128, B],
                    mdt,
                    tag=f"gh1_{h}",
                    name=f"gh1_{h}",
                    addr_space="Shared",
                )
                for h in range(N_AG)
            ]

            h0_loc = []  # 64 chunks [128, B]: full H0 (replicated compute)
            h2_loc = []  # 8 chunks: local H2 shard

            # ---- L0: replicated; 8 waves x (8 out-chunks x 2-step contraction)
            with nc.named_scope("L0"):
                for wave in range(8):
                    wt = wpool.tile([128, 2048], mdt, tag="wt", name="wt0")
                    nc.sync.dma_start(out=wt[:], in_=w0[wave])
                    ps = [
                        pspool.tile([128, B], f32, tag="ps", name=f"ps0_{wave}_{i}")
                        for i in range(8)
                    ]
                    for m8 in range(8):
                        for cc in range(2):
                            nc.tensor.matmul(
                                ps[m8][:],
                                wt[:, cc * 1024 + m8 * 128 : cc * 1024 + (m8 + 1) * 128],
                                x_t[:, cc, :],
                                start=(cc == 0),
                                stop=(cc == 1),
                            )
                    for m8 in range(8):
                        k = wave * 8 + m8
                        h = h0pool.tile([128, B], mdt, tag="h0", name="h0c")
                        drain(h[:], ps[m8][:], b0t[:, k : k + 1], m8)
                        h0_loc.append(h)

            # ---- L1: contract 64 chunks (SBUF h0) in four column quarters
            # so the all-gathers fire at 25/50/75/100% of L1 and overlap the
            # remaining quarters' compute
            with nc.named_scope("L1"):
                for hh in range(N_AG):
                    ps1 = [
                        pspool.tile([128, B], f32, tag="ps", name=f"ps1_{hh}_{i}")
                        for i in range(M_GRP)
                    ]
                    for sc in range(4):
                        wt = wpool.tile([128, 4096], mdt, tag="wt", name="wt1")
                        nc.sync.dma_start(out=wt[:], in_=w1[hh, sc])
                        for c16 in range(16):
                            c = sc * 16 + c16
                            for m in range(M_GRP):
                                nc.tensor.matmul(
                                    ps1[m][:],
                                    wt[:, c16 * 256 + m * 128 : c16 * 256 + (m + 1) * 128],
                                    h0_loc[c][:],
                                    start=(c == 0),
                                    stop=(c == NCH - 1),
                                )
                    for mi in range(M_GRP):
                        m = hh * M_GRP + mi
                        h = spool.tile([128, B], mdt, tag="h1", name=f"h1_{m}")
                        drain(h[:], ps1[mi][:], b1t[:, m : m + 1], mi)
                        nc.gpsimd.dma_start(
                            out=hb1[hh][mi * 128 : (mi + 1) * 128, :], in_=h[:]
                        )
                    nc.gpsimd.collective_compute(
                        "AllGather",
                        mybir.AluOpType.bypass,
                        ins=[hb1[hh][:]],
                        outs=[gh1[hh][:]],
                        replica_groups=RG,
                    )

            # ---- L2: contract 64 chunks (gathered H1), output local shard ----
            with nc.named_scope("L2"):
                ps2 = [
                    pspool.tile([128, B], f32, tag="ps", name=f"ps2_{i}")
                    for i in range(M_LOC)
                ]
                for sc in range(16):
                    wt2 = wpool.tile([128, 4096], mdt, tag="wt", name="wt2")
                    nc.sync.dma_start(out=wt2[:], in_=w2[sc])
                    hh, s_in = divmod(sc, 16 // N_AG)
                    ht = hpool.tile([128, 4, B], mdt, tag="ht", name="ht")
                    nc.gpsimd.dma_start(
                        out=ht[:],
                        in_=gh1[hh][s_in * 512 : (s_in + 1) * 512, :].rearrange(
                            "(c p) n -> p c n", p=128
                        ),
                    )
                    for c4 in range(4):
                        c = sc * 4 + c4
                        for m in range(M_LOC):
                            nc.tensor.matmul(
                                ps2[m][:],
                                wt2[:, c4 * 1024 + m * 128 : c4 * 1024 + (m + 1) * 128],
                                ht[:, c4, :],
                                start=(c == 0),
                                stop=(c == NCH - 1),
                            )
                for m in range(M_LOC):
                    h = h2pool.tile([128, B], mdt, tag="h2", name=f"h2_{m}")
                    drain(h[:], ps2[m][:], b2t[:, m : m + 1], m)
                    h2_loc.append(h)

            # ---- L3: contract local 8 chunks, output partial [256, B] ----
            with nc.named_scope("L3"):
                ps3 = [
                    pspool.tile([128, B], f32, tag="ps", name=f"ps3_{i}")
                    for i in range(2)
                ]
                for cx in range(M_LOC):
                    for mo in range(2):
                        nc.tensor.matmul(
                            ps3[mo][:],
                            w3t[:, cx * F_OUT + mo * 128 : cx * F_OUT + (mo + 1) * 128],
                            h2_loc[cx][:],
                            start=(cx == 0),
                            stop=(cx == M_LOC - 1),
                        )
                for mo in range(2):
                    ot = spool.tile([128, B], f32, tag="ot", name=f"ot_{mo}")
                    nc.vector.tensor_scalar(
                        out=ot[:],
                        in0=ps3[mo][:],
                        scalar1=b3t[:, mo : mo + 1],
                        scalar2=None,
                        op0=mybir.AluOpType.add,
                    )
                    nc.sync.dma_start(out=outp[mo], in_=ot[:])

    nc.compile()
    return nc


def _trap(n, dx):
    w = np.ones(n, np.float64)
    w[0] = 0.5
    w[-1] = 0.5
    return (w * dx).astype(np.float32)


def _prep_inputs(inputs, mm_name):
    if mm_name == "bfloat16":
        import ml_dtypes

        mm_np = ml_dtypes.bfloat16
    else:
        mm_np = np.float32

    x = np.asarray(inputs["x"], np.float32)
    W0 = np.asarray(inputs["W0"], np.float32)
    W1 = np.asarray(inputs["W1"], np.float32)
    W2 = np.asarray(inputs["W2"], np.float32)
    W3 = np.asarray(inputs["W3"], np.float32)
    B0 = np.asarray(inputs["B0"], np.float32)
    B1 = np.asarray(inputs["B1"], np.float32)
    B2 = np.asarray(inputs["B2"], np.float32)
    B3 = np.asarray(inputs["B3"], np.float32)
    tw_in = _trap(QI, DX_IN)
    tw_h = _trap(QH, DX_HID)

    # x -> feature-major [(j p), i], chunked [2, 128, B]
    xT = np.ascontiguousarray(x.reshape(B, F_IN).T).reshape(2, 128, B).astype(mm_np)

    # L0 (replicated): [wave, p, (c f1024)] -> per-partition 8 KiB runs
    Wn0 = (W0 * tw_in[None, None, :, None]).transpose(0, 2, 1, 3).reshape(F_IN, F_HID)
    w0_dev = np.ascontiguousarray(
        Wn0.reshape(2, 128, 8, 1024).transpose(2, 1, 0, 3).reshape(8, 128, 2048)
    ).astype(mm_np)

    order1 = _gh1_order()

    def hidden_mat(W):  # [j,k,p,q] -> [(j p), (k q)] with tw_h folded into rows
        return (W * tw_h[None, None, :, None]).transpose(0, 2, 1, 3).reshape(F_HID, -1)

    Wn1 = hidden_mat(W1)
    Wn2 = hidden_mat(W2)[order1]  # rows follow the gathered-H1 ordering
    Wn3 = hidden_mat(W3)

    per_rank = []
    for r in range(R):
        cols = slice(r * SHARD, (r + 1) * SHARD)  # k in [r*8, r*8+8) natural
        # L1: [quarter, sc4, p, (c16 f256)]; rows (c p), cols (quarter f256)
        w1r = Wn1[:, cols].reshape(4, 16, 128, N_AG, 256)  # [sc, c16, p, q, f]
        w1_dev = np.ascontiguousarray(
            w1r.transpose(3, 0, 2, 1, 4).reshape(N_AG, 4, 128, 4096)
        ).astype(mm_np)
        # L2: [sc16, p, (c4 f1024)]
        w2r = Wn2[:, cols].reshape(16, 4, 128, 1024)  # [sc, c4, p, f]
        w2_dev = np.ascontiguousarray(
            w2r.transpose(0, 2, 1, 3).reshape(16, 128, 4096)
        ).astype(mm_np)
        # L3: rows j in own shard -> [p, (c F_OUT)]
        w3r = Wn3[r * SHARD : (r + 1) * SHARD].reshape(M_LOC, 128, F_OUT)
        w3_dev = np.ascontiguousarray(
            w3r.transpose(1, 0, 2).reshape(128, M_LOC * F_OUT)
        ).astype(mm_np)
        per_rank.append(
            {
                "xT": xT,
                "w0": w0_dev,
                "w1": w1_dev,
                "w2": w2_dev,
                "w3": w3_dev,
                "b0": np.ascontiguousarray(B0.T),
                "b1": np.ascontiguousarray(B1[r * M_LOC : (r + 1) * M_LOC].T),
                "b2": np.ascontiguousarray(B2[r * M_LOC : (r + 1) * M_LOC].T),
                "b3": np.ascontiguousarray((B3.reshape(F_OUT) / R).reshape(2, 128).T),
            }
        )
    return per_rank


def kernel(**inputs):
    global LAST_EXEC_NS, LAST_RESULT
    mm_name = os.environ.get("KERNEL_MM_DT", "float32r")
    trace = os.environ.get("KERNEL_TRACE", "0") == "1"
    if trace:
        _install_ntff_hook()

    from concourse.bass_utils import run_bass_kernel_spmd

    nc = _CACHE.get(mm_name)
    if nc is None:
        nc = _CACHE[mm_name] = _build_nc(mm_name)

    in_maps = _prep_inputs(inputs, mm_name)
    res = run_bass_kernel_spmd(nc, in_maps, core_ids=list(range(R)), trace=trace)
    LAST_EXEC_NS = res.exec_time_ns
    LAST_RESULT = res

    total = np.zeros((2, 128, B), np.float32)
    for r in range(R):
        total += np.asarray(res.results[r]["out"], np.float32)
    # rows are output features (c*QI + q); columns are batch
    return np.ascontiguousarray(
        total.reshape(F_OUT, B).reshape(C, QI, B).transpose(2, 0, 1)
    )
